# revision 53
# baseline (speedup 1.0000x reference)
"""Bior 2x upsampling (zero-interleave + separable 9-tap filter) on 8 TRN2 cores.

Math: y[n] = sum_m h[n+4-2m] x[m] along each spatial axis (SAME zero padding).
Both separable stages are banded matmuls on the TensorEngine:

  stage 1: T1[w, nh] = sum_h X[h, w]  * A[nh, h]   (lhsT = X,  K = h)
  stage 2: Y[nh, nw] = sum_w T1[w, nh] * A[nw, w]  (lhsT = T1, K = w)

with A[n, m] = h[n+4-2m]. A is shift invariant, so every matmul rhs is a
column-slice of CAx[i,j] = h[j+4-2i] (or CBx[i,j] = h[j-256-2i]), shipped
once as a [128, 1040] constant.

Dataflow per core (2 images, "stream" body):
- x arrives as bf16 (host converts: halves the HBM read; ~4e-3 rel err).
- Stage 1 computes t1 in nh-HALVES: each 512-col half needs only 3 of the
  4 input h-tiles, so the first output rows are computable after 3/4 of the
  read. Per half it emits FIVE overlapping 128-row w-windows of t1
  (S2_WINDOWS), so that in stage 2 every output column is covered by exactly
  ONE window matmul (S2_PLAN) - no mostly-zero "corner" matmuls for the
  9-tap halo straddling 128-row K-tile boundaries (stage-2 PE work drops
  1560 -> 1024 cols per 128-row block). Stage-1 itself keeps the 3-MM
  corner structure (its lhsT = x tiles are fixed 128-partition tiles).
- All matmuls run in bf16 (t1 stored bf16): bf16 sustains 1 cycle/row at
  ANY output width, while f32r needs N >= 256 (the sub-256 window MMs would
  run at 1/4 rate in f32r). End-to-end rel err ~7.6e-3 (gate: 2e-2).
- Stage-2 PSUM is two 1-bank tiles per 128-row block (psA/psB): 4-deep
  effective buffering kills wait-on-evac stalls, and the halves evacuate
  concurrently (ACT bank0 / DVE bank1). Stage-1 evacs alternate ACT/DVE
  (GPSIMD cannot read PSUM on TRN2).
- MM order per image: s1(g0), s1(g1), s2(g0), s2(g1) - stage-2's entry
  wait on stage-1 evacs hides behind the second stage-1 half's matmuls.
- y is written to HBM as bf16 (halves the write; host upcasts to f32).
- Timing loop: bodies run in groups of 8 per For_i iteration (the Tile
  loop-reset barrier costs ~7us/iteration; unrolling amortizes it).

has_written semantics (HW-validated): start=True clears the whole BANK's
bits then writes+sets; start=False accumulates where set, overwrites where
not - so disjoint/overlapping column ranges compose with no pre-zeroing.

Sharding: pure data parallel, 2 images per core across 8 cores.
"""

import numpy as np

H_TILDE = np.array([0.03782845550699535, -0.02384946501937986, -0.1106244044184226,
                    0.3774028556126536, 0.8526986790094022, 0.3774028556126537,
                    -0.1106244044184226, -0.02384946501937986, 0.03782845550699535],
                   dtype=np.float32)

B_PER_CORE = 2
N_CORES = 8
H = W = 512
HO = WO = 1024

# "f32r" (fast, ~2e-4 rel err) or "f32" (4x slower matmuls, ~1e-6 rel err)
MM_DTYPE = "f32r"
# "bf16": input shipped/read as bf16 (halves input HBM read; stage-1 MMs in
# bf16 against bf16 constants; stage-2 unchanged). ~3e-3 rel err.
IN_DTYPE = "bf16"
# "bf16": y written to HBM as bf16 (halves output write traffic), host
# upcasts to f32 after gather. Adds <=2^-9 relative rounding on y.
OUT_DTYPE = "bf16"
# "bf16": t1 stored bf16 -> stage-2 MMs in bf16 (2x PE throughput)
T1_DTYPE = "bf16"
EVAC_MODE = "banksplit"
OUT_RING = "sync"
IN_ENG = "scalar"
IN_SPLIT = 1
BODY = "stream"
STAGGERED = False
MM_ORDER = "banks"
PSP_BUFS = 4
XP_BUFS = 2
T1P_BUFS = 12
YP_BUFS = 6
Y_GROUP = 2
PS1_BUFS = 4
PS2_BUFS = 2
S1_EVAC = "alt"
PREFETCH = True
PF_ENG = "scalar"       # ring for the prefetch trigger: "scalar" | "sync"
PF_PRIME_ENG = "scalar"
# "phase": s1g0 s2g0 s1g1 s2g1 (min head latency)
# "s1early": s1g0 s1g1 s2g0 s2g1 (s2-entry evac stall hidden behind s1g1 MMs)
MM_SCHED = "s1early"
# split each stage-2 PSUM block into two 1-bank tiles: 2x effective psum
# buffering (kills the r2/r3 wait-on-evac stalls) + concurrent ACT/DVE evac
PS2_SPLIT = True
# "win": stage-1 emits 5 overlapping 128-row w-windows of t1 per nh-half so
# every stage-2 output column is covered by exactly ONE window MM (no
# mostly-zero corner matmuls): stage-2 PE work drops 1560->1024 cols/block
# at the cost of +1 stage-1 window (24->30 MMs/image).
S2_MODE = "win"
# stage-2 (window, N-range, cax column offset) table; W2 serves two ranges
# split at the PSUM bank boundary (start=True clears per bank)
S2_WINDOWS = [0, 122, 244, 366, 384]
S2_PLAN = [  # (window idx, n0, n1, cax col offset n0-2*w0)
    (0, 0, 252, 0),
    (1, 252, 496, 8),
    (2, 496, 512, 8),
    (2, 512, 740, 24),
    (3, 740, 984, 8),
    (4, 984, 1024, 216),
]
# stage-1 corner MMs at N=4 (their rhs is nonzero in only <=4 cols); bf16
# has no sub-256-N rate cliff, so this saves ~256 wasted cols per corner
S1_NARROW_CORNER = False
# engine for stage-2 psB evac: "dve" | "alt" (alternate DVE/ACT per block,
# rebalancing when DVE's per-copy cost exceeds ACT's)
S2B_EVAC = "dve"
# 5th stage-1 window's psum comes from the ps2a pool (see emit_s1):
# measured WORSE in sim (shifts the wait into stage-2's first psA blocks)
S1_W4_PS2 = False

_CACHE = {}


def _consts():
    """One [128, 1040] f32 constant: CAx | CBx (each [128, 520]).

    CAx[i, j] = h[j + 4 - 2i], CBx[i, j] = h[j - 256 - 2i]. Slices:
      main  rhs aligned at +0   : cax[:, 0:260]   /  cbx[:, 256:516]
      corner rhs (same N=260)   : cax[:, 256:516] /  cbx[:, 0:260]
    """
    h = H_TILDE
    cax = np.zeros((128, 520), dtype=np.float32)
    cbx = np.zeros((128, 520), dtype=np.float32)
    for i in range(128):
        for j in range(520):
            k = j + 4 - 2 * i
            if 0 <= k <= 8:
                cax[i, j] = h[k]
            k = j - 256 - 2 * i
            if 0 <= k <= 8:
                cbx[i, j] = h[k]
    return np.concatenate([cax, cbx], axis=1)


def _split_multiwaits(nc, mybir):
    """walrus here encodes at most ONE sem-wait per instruction; hoist extras
    onto preceding same-engine nops (sequencer order => identical semantics)."""
    ctr = 0
    for fn in nc.m.functions:
        for bb in fn.blocks:
            out, changed = [], False
            for ins in bb.instructions:
                si = ins.sync_info
                if si is not None and len(si.on_wait) > 1:
                    waits = list(si.on_wait)
                    for w in waits[:-1]:
                        ctr += 1
                        nop = mybir.InstNoOp(name=f"wsplit-{ctr}", ins=[], outs=[])
                        nop.engine = ins.engine
                        nop.sync_info = mybir.SyncInfo(on_wait=[w], on_update=[])
                        out.append(nop)
                    si.on_wait = [waits[-1]]
                    changed = True
                out.append(ins)
            if changed:
                bb.instructions = out
    return ctr


def _emit_block(nc, ps, src, mlo, mhi, cax, cbx, f32r, MM_ORDER=None):
    if MM_ORDER is None:
        MM_ORDER = globals()["MM_ORDER"]
    """Emit the 6 uniform [K=128, M=128, N=260] matmuls for one block.

    ps: PSUM [128, 1024]; src: 4 source tiles (partitions = contraction dim);
    mlo:mhi: the 128-wide free-dim slice of the source tiles forming M.
    Corners are full-shape MMs whose rhs is mostly zeros (uniform shape
    keeps the PE pipeline dense; tiny-N MMs measured ~600ns each)."""
    mm = nc.tensor.matmul
    kw = dict(skip_group_check=True)
    if MM_ORDER == "banks":
        mm(ps[:, 0:260], lhsT=src[0][:, mlo:mhi], rhs=cax[:, 0:260],
           start=True, stop=False, **kw)
        mm(ps[:, 252:512], lhsT=src[1][:, mlo:mhi], rhs=cbx[:, 256:516],
           start=False, stop=False, **kw)
        mm(ps[:, 252:512], lhsT=src[2][:, mlo:mhi], rhs=cbx[:, 0:260],
           start=False, stop=False, **kw)
        mm(ps[:, 512:772], lhsT=src[2][:, mlo:mhi], rhs=cax[:, 0:260],
           start=True, stop=False, **kw)
        mm(ps[:, 512:772], lhsT=src[1][:, mlo:mhi], rhs=cax[:, 256:516],
           start=False, stop=False, **kw)
        mm(ps[:, 764:1024], lhsT=src[3][:, mlo:mhi], rhs=cbx[:, 256:516],
           start=False, stop=True, **kw)
    else:  # "paired": same-lhsT MMs adjacent; bank1's first writer is the
           # tile1 corner (start=True overwrites with zeros+corner, then
           # tile2 main accumulates) — identical math via has_written rules
        mm(ps[:, 0:260], lhsT=src[0][:, mlo:mhi], rhs=cax[:, 0:260],
           start=True, stop=False, **kw)
        mm(ps[:, 252:512], lhsT=src[1][:, mlo:mhi], rhs=cbx[:, 256:516],
           start=False, stop=False, **kw)
        mm(ps[:, 512:772], lhsT=src[1][:, mlo:mhi], rhs=cax[:, 256:516],
           start=True, stop=False, **kw)
        mm(ps[:, 252:512], lhsT=src[2][:, mlo:mhi], rhs=cbx[:, 0:260],
           start=False, stop=False, **kw)
        mm(ps[:, 512:772], lhsT=src[2][:, mlo:mhi], rhs=cax[:, 0:260],
           start=False, stop=False, **kw)
        mm(ps[:, 764:1024], lhsT=src[3][:, mlo:mhi], rhs=cbx[:, 256:516],
           start=False, stop=True, **kw)


def _build_program(reps=1, timing_mode=False, loop_n=None, unroll=1,
                   skip_in=False, skip_out=False, skip_compute=False):
    import concourse.bass as bass
    import concourse.mybir as mybir
    import concourse.tile as tile

    f32 = mybir.dt.float32
    dmm = mybir.dt.float32r if MM_DTYPE == "f32r" else f32
    bf16 = mybir.dt.bfloat16
    dt_in = bf16 if IN_DTYPE == "bf16" else dmm
    dt_out = bf16 if OUT_DTYPE == "bf16" else f32
    dt_t1 = bf16 if T1_DTYPE == "bf16" else dmm

    need_c32 = IN_DTYPE != "bf16" or T1_DTYPE != "bf16"

    nc = bass.Bass("TRN2", target_bir_lowering=False, debug=False,
                   num_devices=N_CORES)
    if timing_mode:
        # same dataflow, but keep the big tensors device-internal so the
        # per-call wall isn't dominated by host<->device shipping
        x_d = nc.dram_tensor("x", [B_PER_CORE, H, W], dt_in, kind="Internal")
        y_d = nc.dram_tensor("y", [B_PER_CORE, HO, WO], dt_out, kind="Internal")
        ydummy_d = nc.dram_tensor("ydummy", [1, 4], f32, kind="ExternalOutput")
    else:
        x_d = nc.dram_tensor("x", [B_PER_CORE, H, W], dt_in, kind="ExternalInput")
        y_d = nc.dram_tensor("y", [B_PER_CORE, HO, WO], dt_out, kind="ExternalOutput")
    c_d = (nc.dram_tensor("c", [128, 1040], dmm, kind="ExternalInput")
           if need_c32 else None)
    need_c16 = IN_DTYPE == "bf16" or T1_DTYPE == "bf16"
    c16_d = (nc.dram_tensor("c16", [128, 1040], bf16, kind="ExternalInput")
             if need_c16 else None)

    import contextlib
    with tile.TileContext(nc) as tc:
        with contextlib.ExitStack() as stack:
            constp = stack.enter_context(tc.tile_pool(name="consts", bufs=1))
            xp = stack.enter_context(tc.tile_pool(name="xp", bufs=XP_BUFS))
            t1p = stack.enter_context(tc.tile_pool(name="t1p", bufs=T1P_BUFS))
            yp = stack.enter_context(tc.tile_pool(name="yp", bufs=YP_BUFS))
            if BODY == "stream":
                psp1 = stack.enter_context(
                    tc.tile_pool(name="psp1", bufs=PS1_BUFS, space="PSUM"))
                psp2 = stack.enter_context(
                    tc.tile_pool(name="psp2", bufs=PS2_BUFS, space="PSUM"))
            else:
                psp = stack.enter_context(
                    tc.tile_pool(name="psp", bufs=PSP_BUFS, space="PSUM"))

            cax = cbx = None
            if need_c32:
                c_t = constp.tile([128, 1040], dmm)
                nc.scalar.dma_start(out=c_t[:], in_=c_d.ap())
                cax = c_t[:, 0:520]
                cbx = c_t[:, 520:1040]
            if need_c16:
                c16_t = constp.tile([128, 1040], bf16)
                nc.scalar.dma_start(out=c16_t[:], in_=c16_d.ap())
                cax16, cbx16 = c16_t[:, 0:520], c16_t[:, 520:1040]
            if timing_mode:
                src4 = (c_t[0:1, 0:4].bitcast(f32) if need_c32
                        else c16_t[0:1, 0:8].bitcast(f32))
                nc.sync.dma_start(out=ydummy_d.ap(), in_=src4)
            # stage-1 consts match x dtype; stage-2 consts match t1 dtype
            cax1, cbx1 = (cax16, cbx16) if IN_DTYPE == "bf16" else (cax, cbx)
            s2a, s2b = (cax16, cbx16) if T1_DTYPE == "bf16" else (cax, cbx)

            copy_flip = [0]

            def evac(ps, out_tile):
                mode = EVAC_MODE
                i = copy_flip[0]
                copy_flip[0] += 1
                if mode == "act":
                    nc.scalar.copy(out=out_tile[:], in_=ps[:])
                elif mode == "dve":
                    nc.vector.tensor_copy(out=out_tile[:], in_=ps[:])
                elif mode == "alt":
                    if i % 2 == 0:
                        nc.scalar.copy(out=out_tile[:], in_=ps[:])
                    else:
                        nc.vector.tensor_copy(out=out_tile[:], in_=ps[:])
                elif mode == "rot21":
                    if i % 3 < 2:
                        nc.scalar.copy(out=out_tile[:], in_=ps[:])
                    else:
                        nc.vector.tensor_copy(out=out_tile[:], in_=ps[:])
                elif mode == "banksplit":
                    # ACT bank0, DVE bank1 (different banks, concurrent)
                    nc.scalar.copy(out=out_tile[:, 0:512], in_=ps[:, 0:512])
                    nc.vector.tensor_copy(out=out_tile[:, 512:1024], in_=ps[:, 512:1024])
                else:
                    raise ValueError(mode)

            def body():
                # single 2MB input DMA: one read burst instead of 4 cuts HBM
                # read/write turnaround (measured 28.5 vs 34.8us mixed floor)
                if IN_SPLIT == 1:
                    x_big = xp.tile([128, 2 * 4, W], dmm, tag="x", name="x_big")
                    if not skip_in:
                        nc.scalar.dma_start(
                            out=x_big[:],
                            in_=x_d.ap().rearrange("b (t p) w -> p (b t) w", p=128))
                    else:
                        nc.gpsimd.memset(x_big[:].bitcast(f32), 0.0)
                    xts = [[x_big[:, 4 * b + t, :] for t in range(4)]
                           for b in range(B_PER_CORE)]
                else:
                    # per-image input DMAs, both triggered up front on the same
                    # ring (continuous read burst); image-0 compute only waits
                    # on its own 1MB, halving head latency
                    xts = []
                    for b in range(B_PER_CORE):
                        xb = xp.tile([128, 4, W], dmm, tag=f"x{b}",
                                     name=f"x_{b}")
                        if not skip_in:
                            nc.scalar.dma_start(
                                out=xb[:],
                                in_=x_d.ap()[b].rearrange(
                                    "(t p) w -> p t w", p=128))
                        else:
                            nc.gpsimd.memset(xb[:].bitcast(f32), 0.0)
                        xts.append([xb[:, t, :] for t in range(4)])
                for b in range(B_PER_CORE):
                    xt = xts[b]

                    t1 = []
                    for m in range(4):
                        t1m = t1p.tile([128, 1024], dmm, tag="t1", name=f"t1_{b}_{m}")
                        if not skip_compute:
                            ps = psp.tile([128, 1024], f32, tag="ps", name=f"ps1_{b}_{m}")
                            _emit_block(nc, ps, xt, 128 * m, 128 * (m + 1), cax, cbx, dmm)
                            evac(ps, t1m)
                        t1.append(t1m)

                    for rp in range(8 // Y_GROUP):
                        y_pair = yp.tile([128, Y_GROUP, 1024], f32, tag="y",
                                         name=f"y_{b}_{rp}")
                        for j in range(Y_GROUP):
                            r = Y_GROUP * rp + j
                            if not skip_compute:
                                ps = psp.tile([128, 1024], f32, tag="ps", name=f"ps2_{b}_{r}")
                                _emit_block(nc, ps, t1, 128 * r, 128 * (r + 1), cax, cbx, dmm)
                                evac(ps, y_pair[:, j, :])
                            else:
                                nc.gpsimd.memset(y_pair[:, j, :], 0.0)
                        if not skip_out:
                            eng = nc.sync if (OUT_RING == "sync" or rp % 2 == 0) else nc.scalar
                            eng.dma_start(
                                out=y_d.ap()[b].rearrange(
                                    "(r p) c -> p r c", p=128)[
                                        :, Y_GROUP * rp:Y_GROUP * (rp + 1), :],
                                in_=y_pair[:])

            def emit_half(ps, xt, mlo, mhi, g):
                """Stage-1 nh-half g for one w-slice: t1[w, 512g:512g+512].
                Needs only x h-tiles {0,1,2} (g=0) / {1,2,3} (g=1): the first
                output rows become computable after 3/4 of the input."""
                mm = nc.tensor.matmul
                kw = dict(skip_group_check=True)
                narrow = S1_NARROW_CORNER and IN_DTYPE == "bf16"
                if g == 0:
                    mm(ps[:, 0:260], lhsT=xt[0][:, mlo:mhi], rhs=cax1[:, 0:260],
                       start=True, stop=False, **kw)
                    mm(ps[:, 252:512], lhsT=xt[1][:, mlo:mhi],
                       rhs=cbx1[:, 256:516], start=False, stop=False, **kw)
                    if narrow:
                        mm(ps[:, 508:512], lhsT=xt[2][:, mlo:mhi],
                           rhs=cbx1[:, 256:260], start=False, stop=True, **kw)
                    else:
                        mm(ps[:, 252:512], lhsT=xt[2][:, mlo:mhi],
                           rhs=cbx1[:, 0:260], start=False, stop=True, **kw)
                else:
                    mm(ps[:, 0:260], lhsT=xt[2][:, mlo:mhi], rhs=cax1[:, 0:260],
                       start=True, stop=False, **kw)
                    if narrow:
                        mm(ps[:, 0:4], lhsT=xt[1][:, mlo:mhi],
                           rhs=cax1[:, 256:260], start=False, stop=False, **kw)
                    else:
                        mm(ps[:, 0:260], lhsT=xt[1][:, mlo:mhi],
                           rhs=cax1[:, 256:516], start=False, stop=False, **kw)
                    mm(ps[:, 252:512], lhsT=xt[3][:, mlo:mhi],
                       rhs=cbx1[:, 256:516], start=False, stop=True, **kw)

            s1_flip = [0]

            def evac1(ps, out_tile):
                mode = S1_EVAC
                i = s1_flip[0]
                s1_flip[0] += 1
                if mode == "act":
                    nc.scalar.copy(out=out_tile[:], in_=ps[:])
                elif mode == "dve":
                    nc.vector.tensor_copy(out=out_tile[:], in_=ps[:])
                elif mode == "pool":
                    nc.gpsimd.tensor_copy(out=out_tile[:], in_=ps[:])
                elif mode == "pool+act":
                    if i % 2 == 0:
                        nc.gpsimd.tensor_copy(out=out_tile[:], in_=ps[:])
                    else:
                        nc.scalar.copy(out=out_tile[:], in_=ps[:])
                elif mode == "alt":
                    if i % 2 == 0:
                        nc.scalar.copy(out=out_tile[:], in_=ps[:])
                    else:
                        nc.vector.tensor_copy(out=out_tile[:], in_=ps[:])
                elif mode == "rot12":
                    # 1/3 ACT, 2/3 DVE: relieve the more-loaded ACT engine
                    if i % 3 == 0:
                        nc.scalar.copy(out=out_tile[:], in_=ps[:])
                    else:
                        nc.vector.tensor_copy(out=out_tile[:], in_=ps[:])
                elif mode == "banksplit":
                    nc.scalar.copy(out=out_tile[:, 0:256], in_=ps[:, 0:256])
                    nc.vector.tensor_copy(out=out_tile[:, 256:512],
                                          in_=ps[:, 256:512])
                else:
                    raise ValueError(mode)

            def load_x(eng):
                # per-image input DMAs, both triggered up front (continuous
                # read burst); image-0 compute waits only on its own 1MB
                xts = []
                for b in range(B_PER_CORE):
                    xb = xp.tile([128, 4, W], dt_in, tag=f"x{b}", name=f"x_{b}")
                    if not skip_in:
                        eng.dma_start(
                            out=xb[:],
                            in_=x_d.ap()[b].rearrange("(t p) w -> p t w", p=128))
                    else:
                        nc.gpsimd.memset(xb[:], 0.0)
                    xts.append([xb[:, t, :] for t in range(4)])
                return xts

            def compute_stream(xts):
                for b in range(B_PER_CORE):
                    xt = xts[b]

                    if S2_MODE == "win":
                        s1_slices = [(w0, w0 + 128) for w0 in S2_WINDOWS]
                    else:
                        s1_slices = [(128 * m, 128 * (m + 1)) for m in range(4)]

                    def emit_s1(g):
                        t1g = []
                        for m, (mlo, mhi) in enumerate(s1_slices):
                            t1m = t1p.tile([128, 512], dt_t1, tag="t1",
                                           name=f"t1_{b}_{g}_{m}")
                            if not skip_compute:
                                # the 5th window borrows a stage-2 psum bank:
                                # ps1's 4 buffers then cycle the 4 other
                                # windows exactly, killing the wait-on-evac
                                # stall before w4 (its ps2a buffer's previous
                                # user finished a whole phase earlier)
                                if S1_W4_PS2 and m == 4:
                                    ps = psp2.tile([128, 512], f32, tag="ps2a",
                                                   name=f"ps1w4_{b}_{g}")
                                else:
                                    ps = psp1.tile([128, 512], f32, tag="ps1",
                                                   name=f"ps1_{b}_{g}_{m}")
                                emit_half(ps, xt, mlo, mhi, g)
                                evac1(ps, t1m)
                            t1g.append(t1m)
                        return t1g

                    t1_by_g = {}
                    if MM_SCHED == "s1early":
                        t1_by_g[0] = emit_s1(0)
                        t1_by_g[1] = emit_s1(1)

                    for g in range(2):
                        t1g = t1_by_g[g] if MM_SCHED == "s1early" else emit_s1(g)

                        for rp in range(2):
                            y_pair = yp.tile([128, 2, 1024], dt_out, tag="y",
                                             name=f"y_{b}_{g}_{rp}")
                            for j in range(2):
                                rl = 2 * rp + j
                                if skip_compute:
                                    nc.gpsimd.memset(y_pair[:, j, :], 0.0)
                                elif S2_MODE == "win":
                                    mlo, mhi = 128 * rl, 128 * (rl + 1)
                                    psA = psp2.tile([128, 512], f32, tag="ps2a",
                                                    name=f"ps2a_{b}_{g}_{rl}")
                                    psB = psp2.tile([128, 512], f32, tag="ps2b",
                                                    name=f"ps2b_{b}_{g}_{rl}")
                                    mm = nc.tensor.matmul
                                    kw = dict(skip_group_check=True)
                                    started = set()
                                    for q, n0, n1, off in S2_PLAN:
                                        bank = n0 // 512
                                        ps = psA if bank == 0 else psB
                                        lo = n0 - 512 * bank
                                        mm(ps[:, lo:lo + (n1 - n0)],
                                           lhsT=t1g[q][:, mlo:mhi],
                                           rhs=s2a[:, off:off + (n1 - n0)],
                                           start=bank not in started,
                                           stop=(n1 == 1024), **kw)
                                        started.add(bank)
                                    nc.scalar.copy(out=y_pair[:, j, 0:512],
                                                   in_=psA[:])
                                    i2 = copy_flip[0]
                                    copy_flip[0] += 1
                                    if S2B_EVAC == "alt" and i2 % 2 == 0:
                                        nc.scalar.copy(
                                            out=y_pair[:, j, 512:1024],
                                            in_=psB[:])
                                    else:
                                        nc.vector.tensor_copy(
                                            out=y_pair[:, j, 512:1024],
                                            in_=psB[:])
                                elif PS2_SPLIT:
                                    mlo, mhi = 128 * rl, 128 * (rl + 1)
                                    psA = psp2.tile([128, 512], f32, tag="ps2a",
                                                    name=f"ps2a_{b}_{g}_{rl}")
                                    psB = psp2.tile([128, 512], f32, tag="ps2b",
                                                    name=f"ps2b_{b}_{g}_{rl}")
                                    mm = nc.tensor.matmul
                                    kw = dict(skip_group_check=True)
                                    mm(psA[:, 0:260], lhsT=t1g[0][:, mlo:mhi],
                                       rhs=s2a[:, 0:260], start=True, stop=False, **kw)
                                    mm(psA[:, 252:512], lhsT=t1g[1][:, mlo:mhi],
                                       rhs=s2b[:, 256:516], start=False, stop=False, **kw)
                                    mm(psA[:, 252:512], lhsT=t1g[2][:, mlo:mhi],
                                       rhs=s2b[:, 0:260], start=False, stop=False, **kw)
                                    mm(psB[:, 0:260], lhsT=t1g[2][:, mlo:mhi],
                                       rhs=s2a[:, 0:260], start=True, stop=False, **kw)
                                    mm(psB[:, 0:260], lhsT=t1g[1][:, mlo:mhi],
                                       rhs=s2a[:, 256:516], start=False, stop=False, **kw)
                                    mm(psB[:, 252:512], lhsT=t1g[3][:, mlo:mhi],
                                       rhs=s2b[:, 256:516], start=False, stop=True, **kw)
                                    nc.scalar.copy(out=y_pair[:, j, 0:512],
                                                   in_=psA[:])
                                    nc.vector.tensor_copy(
                                        out=y_pair[:, j, 512:1024], in_=psB[:])
                                else:
                                    ps = psp2.tile([128, 1024], f32, tag="ps2",
                                                   name=f"ps2_{b}_{g}_{rl}")
                                    _emit_block(nc, ps, t1g, 128 * rl,
                                                128 * (rl + 1), s2a, s2b, dmm)
                                    evac(ps, y_pair[:, j, :])
                            if not skip_out:
                                rg = 4 * g + 2 * rp
                                nc.sync.dma_start(
                                    out=y_d.ap()[b].rearrange(
                                        "(r p) c -> p r c", p=128)[
                                            :, rg:rg + 2, :],
                                    in_=y_pair[:])

            pf_state = [None]

            def body_stream(is_last=True):
                """One body. With PREFETCH, trigger the NEXT body's input DMA
                at this body's top: its wait (prev body's stage-1 consumers of
                the recycled x buffer) is met by then, so the read overlaps
                this body's compute + writes instead of serializing behind
                this body's ACT stream."""
                engs = {"scalar": nc.scalar, "sync": nc.sync}
                cur = pf_state[0]
                if cur is None:
                    cur = load_x(engs[PF_PRIME_ENG])
                nxt = None
                if PREFETCH and not is_last:
                    nxt = load_x(engs[PF_ENG])
                compute_stream(cur)
                pf_state[0] = nxt

            if BODY == "stream":
                def emit_group(n):
                    pf_state[0] = None
                    for i in range(n):
                        body_stream(is_last=(i == n - 1))
            else:
                def emit_group(n):
                    for _ in range(n):
                        body()

            if loop_n is not None:
                assert loop_n % unroll == 0
                with tc.For_i(0, loop_n // unroll, 1, staggered_reset=STAGGERED):
                    emit_group(unroll)
            else:
                emit_group(reps)

    _split_multiwaits(nc, mybir)
    return nc


def _const_inputs():
    d = {}
    if IN_DTYPE != "bf16" or T1_DTYPE != "bf16":
        d["c"] = _consts()
    if IN_DTYPE == "bf16" or T1_DTYPE == "bf16":
        import ml_dtypes
        d["c16"] = _consts().astype(ml_dtypes.bfloat16)
    return d


def _get_program():
    if "nc" not in _CACHE:
        _CACHE["nc"] = _build_program()
        _CACHE["consts"] = _const_inputs()
    return _CACHE["nc"], _CACHE["consts"]


def kernel(image_batch: np.ndarray) -> np.ndarray:
    from concourse.bass_utils import run_bass_kernel_spmd

    nc, consts = _get_program()
    x = np.ascontiguousarray(
        np.asarray(image_batch, dtype=np.float32).reshape(16, H, W))
    if IN_DTYPE == "bf16":
        import ml_dtypes
        x = x.astype(ml_dtypes.bfloat16)
    in_maps = [
        {"x": x[B_PER_CORE * k:B_PER_CORE * (k + 1)], **consts}
        for k in range(N_CORES)
    ]
    res = run_bass_kernel_spmd(nc, in_maps, core_ids=list(range(N_CORES)))
    out = np.concatenate([r["y"] for r in res.results], axis=0)
    if out.dtype != np.float32:
        out = out.astype(np.float32)
    return out.reshape(16, HO, WO, 1)



# revision 55
# speedup vs baseline: 1.0936x; 1.0936x over previous
"""Bior 2x upsampling (zero-interleave + separable 9-tap filter) on 8 TRN2 cores.

Math: y[n] = sum_m h[n+4-2m] x[m] along each spatial axis (SAME zero padding).
Both separable stages are banded matmuls on the TensorEngine:

  stage 1: T1[w, nh] = sum_h X[h, w]  * A[nh, h]   (lhsT = X,  K = h)
  stage 2: Y[nh, nw] = sum_w T1[w, nh] * A[nw, w]  (lhsT = T1, K = w)

with A[n, m] = h[n+4-2m]. A is shift invariant, so every matmul rhs is a
column-slice of CAx[i,j] = h[j+4-2i] (or CBx[i,j] = h[j-256-2i]), shipped
once as a [128, 1040] constant.

Dataflow per core (2 images, "stream" body):
- x arrives as bf16 (host converts: halves the HBM read; ~4e-3 rel err).
- Stage 1 computes t1 in nh-HALVES: each 512-col half needs only 3 of the
  4 input h-tiles, so the first output rows are computable after 3/4 of the
  read. Per half it emits FIVE overlapping 128-row w-windows of t1
  (S2_WINDOWS), so that in stage 2 every output column is covered by exactly
  ONE window matmul (S2_PLAN) - no mostly-zero "corner" matmuls for the
  9-tap halo straddling 128-row K-tile boundaries (stage-2 PE work drops
  1560 -> 1024 cols per 128-row block). Stage-1 itself keeps the 3-MM
  corner structure (its lhsT = x tiles are fixed 128-partition tiles).
- All matmuls run in bf16 (t1 stored bf16): bf16 sustains 1 cycle/row at
  ANY output width, while f32r needs N >= 256 (the sub-256 window MMs would
  run at 1/4 rate in f32r). End-to-end rel err ~7.6e-3 (gate: 2e-2).
- Stage-2 PSUM is two 1-bank tiles per 128-row block (psA/psB): 4-deep
  effective buffering kills wait-on-evac stalls, and the halves evacuate
  concurrently (ACT bank0 / DVE bank1). Stage-1 evacs alternate ACT/DVE
  (GPSIMD cannot read PSUM on TRN2).
- MM order per image: s1(g0), s1(g1), s2(g0), s2(g1) - stage-2's entry
  wait on stage-1 evacs hides behind the second stage-1 half's matmuls.
- y is written to HBM as bf16 (halves the write; host upcasts to f32).
- Timing loop: bodies run in groups of 8 per For_i iteration (the Tile
  loop-reset barrier costs ~7us/iteration; unrolling amortizes it).

has_written semantics (HW-validated): start=True clears the whole BANK's
bits then writes+sets; start=False accumulates where set, overwrites where
not - so disjoint/overlapping column ranges compose with no pre-zeroing.

Sharding: pure data parallel, 2 images per core across 8 cores.
"""

import numpy as np

H_TILDE = np.array([0.03782845550699535, -0.02384946501937986, -0.1106244044184226,
                    0.3774028556126536, 0.8526986790094022, 0.3774028556126537,
                    -0.1106244044184226, -0.02384946501937986, 0.03782845550699535],
                   dtype=np.float32)

B_PER_CORE = 2
N_CORES = 8
H = W = 512
HO = WO = 1024

# "f32r" (fast, ~2e-4 rel err) or "f32" (4x slower matmuls, ~1e-6 rel err)
MM_DTYPE = "f32r"
# "bf16": input shipped/read as bf16 (halves input HBM read; stage-1 MMs in
# bf16 against bf16 constants; stage-2 unchanged). ~3e-3 rel err.
IN_DTYPE = "bf16"
# "bf16": y written to HBM as bf16 (halves output write traffic), host
# upcasts to f32 after gather. Adds <=2^-9 relative rounding on y.
OUT_DTYPE = "bf16"
# "bf16": t1 stored bf16 -> stage-2 MMs in bf16 (2x PE throughput)
T1_DTYPE = "bf16"
EVAC_MODE = "banksplit"
OUT_RING = "sync"
IN_ENG = "scalar"
IN_SPLIT = 1
BODY = "stream"
STAGGERED = False
MM_ORDER = "banks"
PSP_BUFS = 4
XP_BUFS = 2
T1P_BUFS = 12
YP_BUFS = 6
Y_GROUP = 2
PS1_BUFS = 4
PS2_BUFS = 2
S1_EVAC = "alt"
PREFETCH = True
# input-DMA triggers ride the SP(sync) ring: the ~0.6us/trigger sequencer
# cost comes off the evac-saturated ACT engine (SP is otherwise idle between
# output-DMA triggers)
PF_ENG = "sync"
PF_PRIME_ENG = "sync"
# "phase": s1g0 s2g0 s1g1 s2g1 (min head latency)
# "s1early": s1g0 s1g1 s2g0 s2g1 (s2-entry evac stall hidden behind s1g1 MMs)
MM_SCHED = "s1early"
# split each stage-2 PSUM block into two 1-bank tiles: 2x effective psum
# buffering (kills the r2/r3 wait-on-evac stalls) + concurrent ACT/DVE evac
PS2_SPLIT = True
# "win": stage-1 emits 5 overlapping 128-row w-windows of t1 per nh-half so
# every stage-2 output column is covered by exactly ONE window MM (no
# mostly-zero corner matmuls): stage-2 PE work drops 1560->1024 cols/block
# at the cost of +1 stage-1 window (24->30 MMs/image).
S2_MODE = "win"
# stage-2 (window, N-range, cax column offset) table; W2 serves two ranges
# split at the PSUM bank boundary (start=True clears per bank)
S2_WINDOWS = [0, 122, 244, 366, 384]
S2_PLAN = [  # (window idx, n0, n1, cax col offset n0-2*w0)
    (0, 0, 252, 0),
    (1, 252, 496, 8),
    (2, 496, 512, 8),
    (2, 512, 740, 24),
    (3, 740, 984, 8),
    (4, 984, 1024, 216),
]
# stage-1 corner MMs at N=4 (their rhs is nonzero in only <=4 cols); bf16
# has no sub-256-N rate cliff, so this saves ~256 wasted cols per corner
S1_NARROW_CORNER = False
# engine for stage-2 psB evac: "dve" | "alt" (alternate DVE/ACT per block,
# rebalancing when DVE's per-copy cost exceeds ACT's)
S2B_EVAC = "dve"
# 5th stage-1 window's psum comes from the ps2a pool (see emit_s1):
# measured WORSE in sim (shifts the wait into stage-2's first psA blocks)
S1_W4_PS2 = False

_CACHE = {}


def _consts():
    """One [128, 1040] f32 constant: CAx | CBx (each [128, 520]).

    CAx[i, j] = h[j + 4 - 2i], CBx[i, j] = h[j - 256 - 2i]. Slices:
      main  rhs aligned at +0   : cax[:, 0:260]   /  cbx[:, 256:516]
      corner rhs (same N=260)   : cax[:, 256:516] /  cbx[:, 0:260]
    """
    h = H_TILDE
    cax = np.zeros((128, 520), dtype=np.float32)
    cbx = np.zeros((128, 520), dtype=np.float32)
    for i in range(128):
        for j in range(520):
            k = j + 4 - 2 * i
            if 0 <= k <= 8:
                cax[i, j] = h[k]
            k = j - 256 - 2 * i
            if 0 <= k <= 8:
                cbx[i, j] = h[k]
    return np.concatenate([cax, cbx], axis=1)


def _split_multiwaits(nc, mybir):
    """walrus here encodes at most ONE sem-wait per instruction; hoist extras
    onto preceding same-engine nops (sequencer order => identical semantics)."""
    ctr = 0
    for fn in nc.m.functions:
        for bb in fn.blocks:
            out, changed = [], False
            for ins in bb.instructions:
                si = ins.sync_info
                if si is not None and len(si.on_wait) > 1:
                    waits = list(si.on_wait)
                    for w in waits[:-1]:
                        ctr += 1
                        nop = mybir.InstNoOp(name=f"wsplit-{ctr}", ins=[], outs=[])
                        nop.engine = ins.engine
                        nop.sync_info = mybir.SyncInfo(on_wait=[w], on_update=[])
                        out.append(nop)
                    si.on_wait = [waits[-1]]
                    changed = True
                out.append(ins)
            if changed:
                bb.instructions = out
    return ctr


def _emit_block(nc, ps, src, mlo, mhi, cax, cbx, f32r, MM_ORDER=None):
    if MM_ORDER is None:
        MM_ORDER = globals()["MM_ORDER"]
    """Emit the 6 uniform [K=128, M=128, N=260] matmuls for one block.

    ps: PSUM [128, 1024]; src: 4 source tiles (partitions = contraction dim);
    mlo:mhi: the 128-wide free-dim slice of the source tiles forming M.
    Corners are full-shape MMs whose rhs is mostly zeros (uniform shape
    keeps the PE pipeline dense; tiny-N MMs measured ~600ns each)."""
    mm = nc.tensor.matmul
    kw = dict(skip_group_check=True)
    if MM_ORDER == "banks":
        mm(ps[:, 0:260], lhsT=src[0][:, mlo:mhi], rhs=cax[:, 0:260],
           start=True, stop=False, **kw)
        mm(ps[:, 252:512], lhsT=src[1][:, mlo:mhi], rhs=cbx[:, 256:516],
           start=False, stop=False, **kw)
        mm(ps[:, 252:512], lhsT=src[2][:, mlo:mhi], rhs=cbx[:, 0:260],
           start=False, stop=False, **kw)
        mm(ps[:, 512:772], lhsT=src[2][:, mlo:mhi], rhs=cax[:, 0:260],
           start=True, stop=False, **kw)
        mm(ps[:, 512:772], lhsT=src[1][:, mlo:mhi], rhs=cax[:, 256:516],
           start=False, stop=False, **kw)
        mm(ps[:, 764:1024], lhsT=src[3][:, mlo:mhi], rhs=cbx[:, 256:516],
           start=False, stop=True, **kw)
    else:  # "paired": same-lhsT MMs adjacent; bank1's first writer is the
           # tile1 corner (start=True overwrites with zeros+corner, then
           # tile2 main accumulates) — identical math via has_written rules
        mm(ps[:, 0:260], lhsT=src[0][:, mlo:mhi], rhs=cax[:, 0:260],
           start=True, stop=False, **kw)
        mm(ps[:, 252:512], lhsT=src[1][:, mlo:mhi], rhs=cbx[:, 256:516],
           start=False, stop=False, **kw)
        mm(ps[:, 512:772], lhsT=src[1][:, mlo:mhi], rhs=cax[:, 256:516],
           start=True, stop=False, **kw)
        mm(ps[:, 252:512], lhsT=src[2][:, mlo:mhi], rhs=cbx[:, 0:260],
           start=False, stop=False, **kw)
        mm(ps[:, 512:772], lhsT=src[2][:, mlo:mhi], rhs=cax[:, 0:260],
           start=False, stop=False, **kw)
        mm(ps[:, 764:1024], lhsT=src[3][:, mlo:mhi], rhs=cbx[:, 256:516],
           start=False, stop=True, **kw)


def _build_program(reps=1, timing_mode=False, loop_n=None, unroll=1,
                   skip_in=False, skip_out=False, skip_compute=False):
    import concourse.bass as bass
    import concourse.mybir as mybir
    import concourse.tile as tile

    f32 = mybir.dt.float32
    dmm = mybir.dt.float32r if MM_DTYPE == "f32r" else f32
    bf16 = mybir.dt.bfloat16
    dt_in = bf16 if IN_DTYPE == "bf16" else dmm
    dt_out = bf16 if OUT_DTYPE == "bf16" else f32
    dt_t1 = bf16 if T1_DTYPE == "bf16" else dmm

    need_c32 = IN_DTYPE != "bf16" or T1_DTYPE != "bf16"

    nc = bass.Bass("TRN2", target_bir_lowering=False, debug=False,
                   num_devices=N_CORES)
    if timing_mode:
        # same dataflow, but keep the big tensors device-internal so the
        # per-call wall isn't dominated by host<->device shipping
        x_d = nc.dram_tensor("x", [B_PER_CORE, H, W], dt_in, kind="Internal")
        y_d = nc.dram_tensor("y", [B_PER_CORE, HO, WO], dt_out, kind="Internal")
        ydummy_d = nc.dram_tensor("ydummy", [1, 4], f32, kind="ExternalOutput")
    else:
        x_d = nc.dram_tensor("x", [B_PER_CORE, H, W], dt_in, kind="ExternalInput")
        y_d = nc.dram_tensor("y", [B_PER_CORE, HO, WO], dt_out, kind="ExternalOutput")
    c_d = (nc.dram_tensor("c", [128, 1040], dmm, kind="ExternalInput")
           if need_c32 else None)
    need_c16 = IN_DTYPE == "bf16" or T1_DTYPE == "bf16"
    c16_d = (nc.dram_tensor("c16", [128, 1040], bf16, kind="ExternalInput")
             if need_c16 else None)

    import contextlib
    with tile.TileContext(nc) as tc:
        with contextlib.ExitStack() as stack:
            constp = stack.enter_context(tc.tile_pool(name="consts", bufs=1))
            xp = stack.enter_context(tc.tile_pool(name="xp", bufs=XP_BUFS))
            t1p = stack.enter_context(tc.tile_pool(name="t1p", bufs=T1P_BUFS))
            yp = stack.enter_context(tc.tile_pool(name="yp", bufs=YP_BUFS))
            if BODY == "stream":
                psp1 = stack.enter_context(
                    tc.tile_pool(name="psp1", bufs=PS1_BUFS, space="PSUM"))
                psp2 = stack.enter_context(
                    tc.tile_pool(name="psp2", bufs=PS2_BUFS, space="PSUM"))
            else:
                psp = stack.enter_context(
                    tc.tile_pool(name="psp", bufs=PSP_BUFS, space="PSUM"))

            cax = cbx = None
            if need_c32:
                c_t = constp.tile([128, 1040], dmm)
                nc.scalar.dma_start(out=c_t[:], in_=c_d.ap())
                cax = c_t[:, 0:520]
                cbx = c_t[:, 520:1040]
            if need_c16:
                c16_t = constp.tile([128, 1040], bf16)
                nc.scalar.dma_start(out=c16_t[:], in_=c16_d.ap())
                cax16, cbx16 = c16_t[:, 0:520], c16_t[:, 520:1040]
            if timing_mode:
                src4 = (c_t[0:1, 0:4].bitcast(f32) if need_c32
                        else c16_t[0:1, 0:8].bitcast(f32))
                nc.sync.dma_start(out=ydummy_d.ap(), in_=src4)
            # stage-1 consts match x dtype; stage-2 consts match t1 dtype
            cax1, cbx1 = (cax16, cbx16) if IN_DTYPE == "bf16" else (cax, cbx)
            s2a, s2b = (cax16, cbx16) if T1_DTYPE == "bf16" else (cax, cbx)

            copy_flip = [0]

            def evac(ps, out_tile):
                mode = EVAC_MODE
                i = copy_flip[0]
                copy_flip[0] += 1
                if mode == "act":
                    nc.scalar.copy(out=out_tile[:], in_=ps[:])
                elif mode == "dve":
                    nc.vector.tensor_copy(out=out_tile[:], in_=ps[:])
                elif mode == "alt":
                    if i % 2 == 0:
                        nc.scalar.copy(out=out_tile[:], in_=ps[:])
                    else:
                        nc.vector.tensor_copy(out=out_tile[:], in_=ps[:])
                elif mode == "rot21":
                    if i % 3 < 2:
                        nc.scalar.copy(out=out_tile[:], in_=ps[:])
                    else:
                        nc.vector.tensor_copy(out=out_tile[:], in_=ps[:])
                elif mode == "banksplit":
                    # ACT bank0, DVE bank1 (different banks, concurrent)
                    nc.scalar.copy(out=out_tile[:, 0:512], in_=ps[:, 0:512])
                    nc.vector.tensor_copy(out=out_tile[:, 512:1024], in_=ps[:, 512:1024])
                else:
                    raise ValueError(mode)

            def body():
                # single 2MB input DMA: one read burst instead of 4 cuts HBM
                # read/write turnaround (measured 28.5 vs 34.8us mixed floor)
                if IN_SPLIT == 1:
                    x_big = xp.tile([128, 2 * 4, W], dmm, tag="x", name="x_big")
                    if not skip_in:
                        nc.scalar.dma_start(
                            out=x_big[:],
                            in_=x_d.ap().rearrange("b (t p) w -> p (b t) w", p=128))
                    else:
                        nc.gpsimd.memset(x_big[:].bitcast(f32), 0.0)
                    xts = [[x_big[:, 4 * b + t, :] for t in range(4)]
                           for b in range(B_PER_CORE)]
                else:
                    # per-image input DMAs, both triggered up front on the same
                    # ring (continuous read burst); image-0 compute only waits
                    # on its own 1MB, halving head latency
                    xts = []
                    for b in range(B_PER_CORE):
                        xb = xp.tile([128, 4, W], dmm, tag=f"x{b}",
                                     name=f"x_{b}")
                        if not skip_in:
                            nc.scalar.dma_start(
                                out=xb[:],
                                in_=x_d.ap()[b].rearrange(
                                    "(t p) w -> p t w", p=128))
                        else:
                            nc.gpsimd.memset(xb[:].bitcast(f32), 0.0)
                        xts.append([xb[:, t, :] for t in range(4)])
                for b in range(B_PER_CORE):
                    xt = xts[b]

                    t1 = []
                    for m in range(4):
                        t1m = t1p.tile([128, 1024], dmm, tag="t1", name=f"t1_{b}_{m}")
                        if not skip_compute:
                            ps = psp.tile([128, 1024], f32, tag="ps", name=f"ps1_{b}_{m}")
                            _emit_block(nc, ps, xt, 128 * m, 128 * (m + 1), cax, cbx, dmm)
                            evac(ps, t1m)
                        t1.append(t1m)

                    for rp in range(8 // Y_GROUP):
                        y_pair = yp.tile([128, Y_GROUP, 1024], f32, tag="y",
                                         name=f"y_{b}_{rp}")
                        for j in range(Y_GROUP):
                            r = Y_GROUP * rp + j
                            if not skip_compute:
                                ps = psp.tile([128, 1024], f32, tag="ps", name=f"ps2_{b}_{r}")
                                _emit_block(nc, ps, t1, 128 * r, 128 * (r + 1), cax, cbx, dmm)
                                evac(ps, y_pair[:, j, :])
                            else:
                                nc.gpsimd.memset(y_pair[:, j, :], 0.0)
                        if not skip_out:
                            eng = nc.sync if (OUT_RING == "sync" or rp % 2 == 0) else nc.scalar
                            eng.dma_start(
                                out=y_d.ap()[b].rearrange(
                                    "(r p) c -> p r c", p=128)[
                                        :, Y_GROUP * rp:Y_GROUP * (rp + 1), :],
                                in_=y_pair[:])

            def emit_half(ps, xt, mlo, mhi, g):
                """Stage-1 nh-half g for one w-slice: t1[w, 512g:512g+512].
                Needs only x h-tiles {0,1,2} (g=0) / {1,2,3} (g=1): the first
                output rows become computable after 3/4 of the input."""
                mm = nc.tensor.matmul
                kw = dict(skip_group_check=True)
                narrow = S1_NARROW_CORNER and IN_DTYPE == "bf16"
                if g == 0:
                    mm(ps[:, 0:260], lhsT=xt[0][:, mlo:mhi], rhs=cax1[:, 0:260],
                       start=True, stop=False, **kw)
                    mm(ps[:, 252:512], lhsT=xt[1][:, mlo:mhi],
                       rhs=cbx1[:, 256:516], start=False, stop=False, **kw)
                    if narrow:
                        mm(ps[:, 508:512], lhsT=xt[2][:, mlo:mhi],
                           rhs=cbx1[:, 256:260], start=False, stop=True, **kw)
                    else:
                        mm(ps[:, 252:512], lhsT=xt[2][:, mlo:mhi],
                           rhs=cbx1[:, 0:260], start=False, stop=True, **kw)
                else:
                    mm(ps[:, 0:260], lhsT=xt[2][:, mlo:mhi], rhs=cax1[:, 0:260],
                       start=True, stop=False, **kw)
                    if narrow:
                        mm(ps[:, 0:4], lhsT=xt[1][:, mlo:mhi],
                           rhs=cax1[:, 256:260], start=False, stop=False, **kw)
                    else:
                        mm(ps[:, 0:260], lhsT=xt[1][:, mlo:mhi],
                           rhs=cax1[:, 256:516], start=False, stop=False, **kw)
                    mm(ps[:, 252:512], lhsT=xt[3][:, mlo:mhi],
                       rhs=cbx1[:, 256:516], start=False, stop=True, **kw)

            s1_flip = [0]

            def evac1(ps, out_tile):
                mode = S1_EVAC
                i = s1_flip[0]
                s1_flip[0] += 1
                if mode == "act":
                    nc.scalar.copy(out=out_tile[:], in_=ps[:])
                elif mode == "dve":
                    nc.vector.tensor_copy(out=out_tile[:], in_=ps[:])
                elif mode == "pool":
                    nc.gpsimd.tensor_copy(out=out_tile[:], in_=ps[:])
                elif mode == "pool+act":
                    if i % 2 == 0:
                        nc.gpsimd.tensor_copy(out=out_tile[:], in_=ps[:])
                    else:
                        nc.scalar.copy(out=out_tile[:], in_=ps[:])
                elif mode == "alt":
                    if i % 2 == 0:
                        nc.scalar.copy(out=out_tile[:], in_=ps[:])
                    else:
                        nc.vector.tensor_copy(out=out_tile[:], in_=ps[:])
                elif mode == "rot12":
                    # 1/3 ACT, 2/3 DVE: relieve the more-loaded ACT engine
                    if i % 3 == 0:
                        nc.scalar.copy(out=out_tile[:], in_=ps[:])
                    else:
                        nc.vector.tensor_copy(out=out_tile[:], in_=ps[:])
                elif mode == "a3d2":
                    # 3/5 ACT, 2/5 DVE (6:4 per image): balances engine TIME
                    # (ACT is faster per element) once DMA triggers are off ACT
                    if i % 5 in (0, 2, 4):
                        nc.scalar.copy(out=out_tile[:], in_=ps[:])
                    else:
                        nc.vector.tensor_copy(out=out_tile[:], in_=ps[:])
                elif mode == "banksplit":
                    nc.scalar.copy(out=out_tile[:, 0:256], in_=ps[:, 0:256])
                    nc.vector.tensor_copy(out=out_tile[:, 256:512],
                                          in_=ps[:, 256:512])
                else:
                    raise ValueError(mode)

            def load_x(eng):
                # per-image input DMAs, both triggered up front (continuous
                # read burst); image-0 compute waits only on its own 1MB
                xts = []
                for b in range(B_PER_CORE):
                    xb = xp.tile([128, 4, W], dt_in, tag=f"x{b}", name=f"x_{b}")
                    if not skip_in:
                        eng.dma_start(
                            out=xb[:],
                            in_=x_d.ap()[b].rearrange("(t p) w -> p t w", p=128))
                    else:
                        nc.gpsimd.memset(xb[:], 0.0)
                    xts.append([xb[:, t, :] for t in range(4)])
                return xts

            def compute_stream(xts):
                for b in range(B_PER_CORE):
                    xt = xts[b]

                    if S2_MODE == "win":
                        s1_slices = [(w0, w0 + 128) for w0 in S2_WINDOWS]
                    else:
                        s1_slices = [(128 * m, 128 * (m + 1)) for m in range(4)]

                    def emit_s1(g):
                        t1g = []
                        for m, (mlo, mhi) in enumerate(s1_slices):
                            t1m = t1p.tile([128, 512], dt_t1, tag="t1",
                                           name=f"t1_{b}_{g}_{m}")
                            if not skip_compute:
                                # the 5th window borrows a stage-2 psum bank:
                                # ps1's 4 buffers then cycle the 4 other
                                # windows exactly, killing the wait-on-evac
                                # stall before w4 (its ps2a buffer's previous
                                # user finished a whole phase earlier)
                                if S1_W4_PS2 and m == 4:
                                    ps = psp2.tile([128, 512], f32, tag="ps2a",
                                                   name=f"ps1w4_{b}_{g}")
                                else:
                                    ps = psp1.tile([128, 512], f32, tag="ps1",
                                                   name=f"ps1_{b}_{g}_{m}")
                                emit_half(ps, xt, mlo, mhi, g)
                                evac1(ps, t1m)
                            t1g.append(t1m)
                        return t1g

                    t1_by_g = {}
                    if MM_SCHED == "s1early":
                        t1_by_g[0] = emit_s1(0)
                        t1_by_g[1] = emit_s1(1)

                    for g in range(2):
                        t1g = t1_by_g[g] if MM_SCHED == "s1early" else emit_s1(g)

                        for rp in range(2):
                            y_pair = yp.tile([128, 2, 1024], dt_out, tag="y",
                                             name=f"y_{b}_{g}_{rp}")
                            for j in range(2):
                                rl = 2 * rp + j
                                if skip_compute:
                                    nc.gpsimd.memset(y_pair[:, j, :], 0.0)
                                elif S2_MODE == "win":
                                    mlo, mhi = 128 * rl, 128 * (rl + 1)
                                    psA = psp2.tile([128, 512], f32, tag="ps2a",
                                                    name=f"ps2a_{b}_{g}_{rl}")
                                    psB = psp2.tile([128, 512], f32, tag="ps2b",
                                                    name=f"ps2b_{b}_{g}_{rl}")
                                    mm = nc.tensor.matmul
                                    kw = dict(skip_group_check=True)
                                    started = set()
                                    for q, n0, n1, off in S2_PLAN:
                                        bank = n0 // 512
                                        ps = psA if bank == 0 else psB
                                        lo = n0 - 512 * bank
                                        mm(ps[:, lo:lo + (n1 - n0)],
                                           lhsT=t1g[q][:, mlo:mhi],
                                           rhs=s2a[:, off:off + (n1 - n0)],
                                           start=bank not in started,
                                           stop=(n1 == 1024), **kw)
                                        started.add(bank)
                                    nc.scalar.copy(out=y_pair[:, j, 0:512],
                                                   in_=psA[:])
                                    i2 = copy_flip[0]
                                    copy_flip[0] += 1
                                    if S2B_EVAC == "alt" and i2 % 2 == 0:
                                        nc.scalar.copy(
                                            out=y_pair[:, j, 512:1024],
                                            in_=psB[:])
                                    else:
                                        nc.vector.tensor_copy(
                                            out=y_pair[:, j, 512:1024],
                                            in_=psB[:])
                                elif PS2_SPLIT:
                                    mlo, mhi = 128 * rl, 128 * (rl + 1)
                                    psA = psp2.tile([128, 512], f32, tag="ps2a",
                                                    name=f"ps2a_{b}_{g}_{rl}")
                                    psB = psp2.tile([128, 512], f32, tag="ps2b",
                                                    name=f"ps2b_{b}_{g}_{rl}")
                                    mm = nc.tensor.matmul
                                    kw = dict(skip_group_check=True)
                                    mm(psA[:, 0:260], lhsT=t1g[0][:, mlo:mhi],
                                       rhs=s2a[:, 0:260], start=True, stop=False, **kw)
                                    mm(psA[:, 252:512], lhsT=t1g[1][:, mlo:mhi],
                                       rhs=s2b[:, 256:516], start=False, stop=False, **kw)
                                    mm(psA[:, 252:512], lhsT=t1g[2][:, mlo:mhi],
                                       rhs=s2b[:, 0:260], start=False, stop=False, **kw)
                                    mm(psB[:, 0:260], lhsT=t1g[2][:, mlo:mhi],
                                       rhs=s2a[:, 0:260], start=True, stop=False, **kw)
                                    mm(psB[:, 0:260], lhsT=t1g[1][:, mlo:mhi],
                                       rhs=s2a[:, 256:516], start=False, stop=False, **kw)
                                    mm(psB[:, 252:512], lhsT=t1g[3][:, mlo:mhi],
                                       rhs=s2b[:, 256:516], start=False, stop=True, **kw)
                                    nc.scalar.copy(out=y_pair[:, j, 0:512],
                                                   in_=psA[:])
                                    nc.vector.tensor_copy(
                                        out=y_pair[:, j, 512:1024], in_=psB[:])
                                else:
                                    ps = psp2.tile([128, 1024], f32, tag="ps2",
                                                   name=f"ps2_{b}_{g}_{rl}")
                                    _emit_block(nc, ps, t1g, 128 * rl,
                                                128 * (rl + 1), s2a, s2b, dmm)
                                    evac(ps, y_pair[:, j, :])
                            if not skip_out:
                                rg = 4 * g + 2 * rp
                                nc.sync.dma_start(
                                    out=y_d.ap()[b].rearrange(
                                        "(r p) c -> p r c", p=128)[
                                            :, rg:rg + 2, :],
                                    in_=y_pair[:])

            pf_state = [None]

            def body_stream(is_last=True):
                """One body. With PREFETCH, trigger the NEXT body's input DMA
                at this body's top: its wait (prev body's stage-1 consumers of
                the recycled x buffer) is met by then, so the read overlaps
                this body's compute + writes instead of serializing behind
                this body's ACT stream."""
                engs = {"scalar": nc.scalar, "sync": nc.sync}
                cur = pf_state[0]
                if cur is None:
                    cur = load_x(engs[PF_PRIME_ENG])
                nxt = None
                if PREFETCH and not is_last:
                    nxt = load_x(engs[PF_ENG])
                compute_stream(cur)
                pf_state[0] = nxt

            if BODY == "stream":
                def emit_group(n):
                    pf_state[0] = None
                    for i in range(n):
                        body_stream(is_last=(i == n - 1))
            else:
                def emit_group(n):
                    for _ in range(n):
                        body()

            if loop_n is not None:
                assert loop_n % unroll == 0
                with tc.For_i(0, loop_n // unroll, 1, staggered_reset=STAGGERED):
                    emit_group(unroll)
            else:
                emit_group(reps)

    _split_multiwaits(nc, mybir)
    return nc


def _const_inputs():
    d = {}
    if IN_DTYPE != "bf16" or T1_DTYPE != "bf16":
        d["c"] = _consts()
    if IN_DTYPE == "bf16" or T1_DTYPE == "bf16":
        import ml_dtypes
        d["c16"] = _consts().astype(ml_dtypes.bfloat16)
    return d


def _get_program():
    if "nc" not in _CACHE:
        _CACHE["nc"] = _build_program()
        _CACHE["consts"] = _const_inputs()
    return _CACHE["nc"], _CACHE["consts"]


def kernel(image_batch: np.ndarray) -> np.ndarray:
    from concourse.bass_utils import run_bass_kernel_spmd

    nc, consts = _get_program()
    x = np.ascontiguousarray(
        np.asarray(image_batch, dtype=np.float32).reshape(16, H, W))
    if IN_DTYPE == "bf16":
        import ml_dtypes
        x = x.astype(ml_dtypes.bfloat16)
    in_maps = [
        {"x": x[B_PER_CORE * k:B_PER_CORE * (k + 1)], **consts}
        for k in range(N_CORES)
    ]
    res = run_bass_kernel_spmd(nc, in_maps, core_ids=list(range(N_CORES)))
    out = np.concatenate([r["y"] for r in res.results], axis=0)
    if out.dtype != np.float32:
        out = out.astype(np.float32)
    return out.reshape(16, HO, WO, 1)

